# revision 1
# baseline (speedup 1.0000x reference)
"""Single-head causal self-attention on 8 Trainium2 NeuronCores (Bass/Tile).

Problem: x [1024, 256, 384], Wq/Wk/Wv [384, 64] ->
  q,k,v = x@W;  wei = softmax(mask(q k^T / sqrt(384)));  out = wei @ v
Output: [1024, 256, 64] fp32.

Strategy (data-parallel over batch, 128 batches per core):
  - Host pre-transposes x to xT[b, p, c, t] = x[b, t, 128c+p] so the
    contraction dim (C=384, in 3 chunks of 128) lands on SBUF partitions
    with fully contiguous 1KB DMA rows.
  - Per batch, all matmuls run in fp32r (1 cycle/row when moving dim >= 256):
      qk   [128,256] = [Wq|Wk]^T x^T        (3-chunk accumulation)
      vT   [64,256]  = Wv^T x^T             (3-chunk accumulation)
      v    [128,128] = PE-transpose(vT)     (two 64x128 transposes)
      weiT [s,t]     = k q^T                (2 s-halves, K=64)
      P    = exp(weiT/sqrt(384)) * causal   (no max-subtraction: |wei/19.6|<~3)
      outT [65,256]  = [1|v]^T P            (ones col -> row 0 = softmax denom)
      out  = outT[1:65] * broadcast(1/denom)  (broadcast via K=1 matmul)
  - Causal structure: s-half0 is fully valid for t>=128 (mask only the
    diagonal 128x128 block); s-half1 is all-invalid for t<128 (left half of
    P1 kept at a persistent 0), diag-masked for t>=128.
  - Output written as outT [b, h, t]; host transposes back to [b, t, h].
"""

import os
from contextlib import ExitStack

import numpy as np

import concourse.bass as bass
import concourse.bacc as bacc
import concourse.tile as tile
from concourse import mybir
from concourse.bass_utils import run_bass_kernel_spmd

N_CORES = 8
B = 1024
T = 256
C = 384
H = 64
BPC = B // N_CORES  # 128 batches per core
NCHUNK = C // 128  # 3
SCALE = float(C) ** -0.5

F32 = mybir.dt.float32
F32R = mybir.dt.float32r


def r(ap):
    """Bitcast an fp32 AP to fp32r for full-rate matmul streaming."""
    return ap.bitcast(F32R)


def build_nc(bpc: int = BPC):
    nc = bacc.Bacc(
        "TRN2", target_bir_lowering=False, debug=False, num_devices=N_CORES
    )

    xT = nc.dram_tensor("xT", [bpc, 128, NCHUNK, T], F32R, kind="ExternalInput").ap()
    wqk = nc.dram_tensor("wqk", [128, NCHUNK, 128], F32R, kind="ExternalInput").ap()
    wv = nc.dram_tensor("wv", [128, NCHUNK, H], F32R, kind="ExternalInput").ap()
    mask = nc.dram_tensor("mask", [128, 128], F32, kind="ExternalInput").ap()
    eye = nc.dram_tensor("eye", [H, H], F32R, kind="ExternalInput").ap()
    ones = nc.dram_tensor("ones", [1, H], F32R, kind="ExternalInput").ap()
    outT = nc.dram_tensor("outT", [bpc, H, T], F32, kind="ExternalOutput").ap()

    with ExitStack() as ctx:
        tc = ctx.enter_context(tile.TileContext(nc))

        const = ctx.enter_context(tc.tile_pool(name="const", bufs=1))
        wqk_sb = const.tile([128, NCHUNK, 128], F32R, tag="wqk")
        nc.sync.dma_start(wqk_sb[:], wqk)
        wv_sb = const.tile([128, NCHUNK, H], F32R, tag="wv")
        nc.sync.dma_start(wv_sb[:], wv)
        mask_sb = const.tile([128, 128], F32, tag="mask")
        nc.sync.dma_start(mask_sb[:], mask)
        eye_sb = const.tile([H, H], F32R, tag="eye")
        nc.sync.dma_start(eye_sb[:], eye)
        ones_sb = const.tile([1, H], F32R, tag="ones")
        nc.sync.dma_start(ones_sb[:], ones)

        # Persistent double-buffered tiles with preset regions that survive
        # across iterations: v_aug ones-columns (0 and 65) and P1's zero
        # left half (the all-invalid causal block).
        NSLOT = 2
        vaug = []
        p1s = []
        for i in range(NSLOT):
            v_t = const.tile([128, 131], F32R, tag=f"vaug{i}")
            nc.gpsimd.memset(v_t[:, 64:65].bitcast(F32), 1.0)
            nc.gpsimd.memset(v_t[:, 129:130].bitcast(F32), 1.0)
            vaug.append(v_t)
            p_t = const.tile([128, T], F32R, tag=f"p1_{i}")
            nc.gpsimd.memset(p_t[:, 0:128].bitcast(F32), 0.0)
            p1s.append(p_t)

        xt_pool = ctx.enter_context(tc.tile_pool(name="xt", bufs=4))
        sb_pool = ctx.enter_context(tc.tile_pool(name="sb", bufs=2))
        psa_pool = ctx.enter_context(tc.tile_pool(name="psa", bufs=2, space="PSUM"))
        psb_pool = ctx.enter_context(tc.tile_pool(name="psb", bufs=2, space="PSUM"))
        psc_pool = ctx.enter_context(tc.tile_pool(name="psc", bufs=2, space="PSUM"))
        psd_pool = ctx.enter_context(tc.tile_pool(name="psd", bufs=2, space="PSUM"))

        for b in range(bpc):
            slot = b % NSLOT
            v_sb = vaug[slot]
            p1 = p1s[slot]

            xt = xt_pool.tile([128, NCHUNK, T], F32R, tag="xt")
            nc.sync.dma_start(xt[:], xT[b])

            # qk^T [128, 256] (q heads on partitions 0:64, k heads 64:128)
            # and v^T [64, 256], both accumulated over the 3 C-chunks.
            ps_a = psa_pool.tile([128, 512], F32, tag="psa")
            for c in range(NCHUNK):
                nc.tensor.matmul(
                    ps_a[:, 0:T],
                    lhsT=r(wqk_sb[:, c, :]),
                    rhs=r(xt[:, c, :]),
                    start=(c == 0),
                    stop=(c == NCHUNK - 1),
                )
            for c in range(NCHUNK):
                nc.tensor.matmul(
                    ps_a[0:H, T : T + T],
                    lhsT=r(wv_sb[:, c, :]),
                    rhs=r(xt[:, c, :]),
                    start=(c == 0),
                    stop=(c == NCHUNK - 1),
                )

            # q/k copied to separate base-0 tiles (matmul requires lhsT and
            # rhs at the same SBUF base partition).
            q_sb = sb_pool.tile([H, T], F32R, tag="q")
            nc.scalar.copy(q_sb[:], ps_a[0:H, 0:T])
            k_sb = sb_pool.tile([H, T], F32R, tag="k")
            nc.scalar.copy(k_sb[:], ps_a[H:128, 0:T])
            vt_sb = sb_pool.tile([H, T], F32R, tag="vt")
            nc.scalar.copy(vt_sb[:], ps_a[0:H, T : T + T])

            # v [s, h] via two PE transposes of vT s-halves.
            ps_b = psb_pool.tile([128, 128], F32, tag="psb")
            nc.tensor.transpose(r(ps_b[:, 0:64]), r(vt_sb[:, 0:128]), r(eye_sb[:]))
            nc.tensor.transpose(r(ps_b[:, 64:128]), r(vt_sb[:, 128:256]), r(eye_sb[:]))
            # One strided copy drops both halves into v_aug at cols 0:64 and
            # 65:129 (cols 64 and 129 hold the persistent ones).
            dst = v_sb[:, 0:130].rearrange("p (two f) -> p two f", two=2)[:, :, 0:64]
            src = ps_b[:, 0:128].rearrange("p (two f) -> p two f", two=2)
            nc.vector.tensor_copy(dst, src)

            # weiT[s, t] = k q^T for both s-halves (K = 64 heads).
            ps_c = psc_pool.tile([128, 512], F32, tag="psc")
            nc.tensor.matmul(
                ps_c[:, 0:T],
                lhsT=r(k_sb[:, 0:128]),
                rhs=r(q_sb[:]),
                start=True,
                stop=True,
            )
            nc.tensor.matmul(
                ps_c[:, T : T + T],
                lhsT=r(k_sb[:, 128:256]),
                rhs=r(q_sb[:]),
                start=True,
                stop=True,
            )

            # P = exp(weiT * scale); no max-subtraction needed (|arg| < ~3).
            p0 = sb_pool.tile([128, T], F32R, tag="p0")
            nc.scalar.activation(
                p0[:], ps_c[:, 0:T], mybir.ActivationFunctionType.Exp, scale=SCALE
            )
            nc.scalar.activation(
                p1[:, 128:256],
                ps_c[:, T + 128 : T + 256],
                mybir.ActivationFunctionType.Exp,
                scale=SCALE,
            )
            # Causal mask on the two diagonal blocks (GPSIMD, off DVE/ACT).
            nc.gpsimd.tensor_mul(p0[:, 0:128], p0[:, 0:128], mask_sb[:])
            nc.gpsimd.tensor_mul(p1[:, 128:256], p1[:, 128:256], mask_sb[:])

            # outT[65, 256]: row 64 = softmax denominator (ones columns),
            # rows 0:64 = unnormalized out^T. Accumulate both s-halves.
            ps_d = psd_pool.tile([128, 512], F32, tag="psd")
            nc.tensor.matmul(
                ps_d[0:65, 0:T],
                lhsT=r(v_sb[:, 0:65]),
                rhs=r(p0[:]),
                start=True,
                stop=False,
            )
            nc.tensor.matmul(
                ps_d[0:65, 0:T],
                lhsT=r(v_sb[:, 65:130]),
                rhs=r(p1[:]),
                start=False,
                stop=True,
            )

            recip = sb_pool.tile([1, T], F32R, tag="recip")
            with nc.allow_low_precision(reason="softmax denom reciprocal to f32r"):
                nc.vector.reciprocal(recip[:], ps_d[64:65, 0:T])
            # Broadcast 1/denom across 64 partitions via K=1 matmul.
            nc.tensor.matmul(
                ps_d[0:H, T : T + T],
                lhsT=r(ones_sb[:]),
                rhs=r(recip[:]),
                start=True,
                stop=True,
            )
            bc_sb = sb_pool.tile([H, T], F32, tag="bc")
            nc.scalar.copy(bc_sb[:], ps_d[0:H, T : T + T])
            out_sb = sb_pool.tile([H, T], F32, tag="out")
            nc.vector.tensor_mul(out_sb[:], ps_d[0:H, 0:T], bc_sb[:])
            nc.gpsimd.dma_start(outT[b], out_sb[:])

    nc.finalize()  # run Bacc passes (reg alloc, wait splitting) for BIR export
    return nc


def _host_inputs(x, Wq, Wk, Wv):
    B_, T_, C_ = x.shape
    assert (B_, T_, C_) == (B, T, C), (B_, T_, C_)
    xh = np.ascontiguousarray(
        x.reshape(B, T, NCHUNK, 128).transpose(0, 3, 2, 1), dtype=np.float32
    )  # [B, 128, 3, T];  xh[b, p, c, t] == x[b, t, 128c+p]
    wqk_h = np.ascontiguousarray(
        np.concatenate([Wq, Wk], axis=1).reshape(NCHUNK, 128, 128).transpose(1, 0, 2),
        dtype=np.float32,
    )
    wv_h = np.ascontiguousarray(
        Wv.reshape(NCHUNK, 128, H).transpose(1, 0, 2), dtype=np.float32
    )
    mask_h = np.triu(np.ones((128, 128), dtype=np.float32))
    eye_h = np.eye(H, dtype=np.float32)
    ones_h = np.ones((1, H), dtype=np.float32)
    return xh, wqk_h, wv_h, mask_h, eye_h, ones_h


def kernel(x, Wq, Wk, Wv):
    x = np.asarray(x, dtype=np.float32)
    Wq = np.asarray(Wq, dtype=np.float32)
    Wk = np.asarray(Wk, dtype=np.float32)
    Wv = np.asarray(Wv, dtype=np.float32)

    xh, wqk_h, wv_h, mask_h, eye_h, ones_h = _host_inputs(x, Wq, Wk, Wv)

    nc = build_nc(BPC)
    in_maps = [
        {
            "xT": xh[i * BPC : (i + 1) * BPC],
            "wqk": wqk_h,
            "wv": wv_h,
            "mask": mask_h,
            "eye": eye_h,
            "ones": ones_h,
        }
        for i in range(N_CORES)
    ]
    res = run_bass_kernel_spmd(nc, in_maps, list(range(N_CORES)))
    outT = np.concatenate([res.results[i]["outT"] for i in range(N_CORES)], axis=0)
    return np.ascontiguousarray(outT.transpose(0, 2, 1))



# revision 6
# speedup vs baseline: 1.5425x; 1.5425x over previous
"""Single-head causal self-attention on 8 Trainium2 NeuronCores (Bass/Tile).

Problem: x [1024, 256, 384], Wq/Wk/Wv [384, 64] ->
  q,k,v = x@W;  wei = softmax(mask(q k^T / sqrt(384)));  out = wei @ v
Output: [1024, 256, 64] fp32.

Strategy (data-parallel over batch, 128 batches = 64 pairs per core):
  - Layout-by-stationarity: every tensor is produced in exactly the layout
    its consumer needs, so there are NO transposes anywhere.
      q^T,k^T [h, t]  = W-stationary matmul   (lhsT = [Wq|Wk] chunk)
      v       [s, h]  = x-stationary matmul   (lhsT = x^T chunk)
      P       [s, t]  = k-stationary matmul   (lhsT = k^T s-half) + exp
      out     [t, h]  = P-stationary matmul   (lhsT = P block, rhs = [v|1])
  - The ones column appended to v makes out-MM row emit the softmax
    denominator per OUTPUT PARTITION (= token), so normalization is one
    cheap reciprocal [128, 4, 1] + one broadcast tensor_mul per pair.
  - Causal structure at 128-block granularity: tokens t<128 never touch
    s-half1, so wei needs only 3 of 4 blocks (A=s0xt0 diag, B=s0xt1 full,
    C=s1xt1 diag); exp covers [128, 384]; mask = [triu|ones|triu].
  - Pair batching: 2 batches per iteration; qk projection streams N=512,
    elementwise ops fused across the pair (recip FD=4, one norm op).
  - All matmuls fp32r (full-rate streaming); PSUM tiles padded to full
    banks (4 pools x 2 bufs = 8 banks exactly).
"""

import os
from contextlib import ExitStack

import numpy as np

import concourse.bass as bass
import concourse.bacc as bacc
import concourse.tile as tile
from concourse import mybir
from concourse.bass import broadcast_tensor_aps
from concourse.bass_utils import run_bass_kernel_spmd

N_CORES = 8
B = 1024
T = 256
C = 384
H = 64
BPC = B // N_CORES  # 128 batches per core
NPAIR = BPC // 2  # 64 pairs per core
NCHUNK = C // 128  # 3
SCALE = float(C) ** -0.5

F32 = mybir.dt.float32
F32R = mybir.dt.float32r


def r(ap):
    """Bitcast an fp32 AP to fp32r for full-rate matmul streaming."""
    return ap.bitcast(F32R)


def build_nc(npair: int = NPAIR):
    nc = bacc.Bacc(
        "TRN2", target_bir_lowering=False, debug=False, num_devices=N_CORES
    )

    # xT[pr, p, c, 256*b2 + t] = x[2*pr + b2, t, 128*c + p]
    xT = nc.dram_tensor("xT", [npair, 128, NCHUNK, 2 * T], F32R, kind="ExternalInput").ap()
    wqk = nc.dram_tensor("wqk", [128, NCHUNK, 128], F32R, kind="ExternalInput").ap()
    wv = nc.dram_tensor("wv", [128, NCHUNK, H], F32R, kind="ExternalInput").ap()
    # [triu | triu] for the A and C (diagonal) block columns of P
    mask = nc.dram_tensor("mask", [128, 256], F32, kind="ExternalInput").ap()
    # outH[pr, p, 64*g + h] = out[2*pr + g//2, 128*(g%2) + p, h]
    outH = nc.dram_tensor("outH", [npair, 128, 4 * H], F32, kind="ExternalOutput").ap()

    with ExitStack() as ctx:
        tc = ctx.enter_context(tile.TileContext(nc))

        const = ctx.enter_context(tc.tile_pool(name="const", bufs=1))
        wqk_sb = const.tile([128, NCHUNK, 128], F32R, tag="wqk")
        nc.sync.dma_start(wqk_sb[:], wqk)
        wv_sb = const.tile([128, NCHUNK, H], F32R, tag="wv")
        nc.sync.dma_start(wv_sb[:], wv)
        mask_sb = const.tile([128, 256], F32, tag="mask")
        nc.sync.dma_start(mask_sb[:], mask)

        # Persistent [v | 1] rhs tiles: 4 groups (b0s0, b0s1, b1s0, b1s1),
        # each [128, 66]: ones column at col 64, col 65 pad (fp32r moving
        # operands need an even free dim; 66*4B keeps PSUM offsets aligned).
        GW = H + 2
        NSLOT = 3
        vaug = []
        for i in range(NSLOT):
            v_t = const.tile([128, 4, GW], F32R, tag=f"vaug{i}")
            nc.gpsimd.memset(v_t[:, :, H : H + 2].bitcast(F32), 1.0)
            vaug.append(v_t)

        xt_pool = ctx.enter_context(tc.tile_pool(name="xt", bufs=3))
        q_pool = ctx.enter_context(tc.tile_pool(name="qp", bufs=2))
        k_pool = ctx.enter_context(tc.tile_pool(name="kp", bufs=2))
        p_pool = ctx.enter_context(tc.tile_pool(name="pp", bufs=3))
        o_pool = ctx.enter_context(tc.tile_pool(name="op", bufs=2))
        r_pool = ctx.enter_context(tc.tile_pool(name="rp", bufs=2))

        psqk_pool = ctx.enter_context(tc.tile_pool(name="psqk", bufs=2, space="PSUM"))
        psv_pool = ctx.enter_context(tc.tile_pool(name="psv", bufs=2, space="PSUM"))
        psc_pool = ctx.enter_context(tc.tile_pool(name="psc", bufs=2, space="PSUM"))
        pso_pool = ctx.enter_context(tc.tile_pool(name="pso", bufs=2, space="PSUM"))

        for pr in range(npair):
            xt = xt_pool.tile([128, NCHUNK, 2 * T], F32R, tag="xt")
            nc.sync.dma_start(xt[:], xT[pr])

            # q^T|k^T for the pair: [128 qk-dims, 512 tokens]
            ps_qk = psqk_pool.tile([128, 512], F32, tag="psqk")
            for c in range(NCHUNK):
                nc.tensor.matmul(
                    ps_qk[:],
                    lhsT=r(wqk_sb[:, c, :]),
                    rhs=r(xt[:, c, :]),
                    start=(c == 0),
                    stop=(c == NCHUNK - 1),
                )
            q_sb = q_pool.tile([H, 512], F32R, tag="q")
            nc.scalar.copy(q_sb[:], ps_qk[0:H, :])
            k_sb = k_pool.tile([H, 512], F32R, tag="k")
            nc.scalar.copy(k_sb[:], ps_qk[H:128, :])

            # v [s, h] directly via x-stationary matmuls: group g = 2*b2 + s
            ps_v = psv_pool.tile([128, 512], F32, tag="psv")
            for g in range(4):
                b2, s = divmod(g, 2)
                t0 = b2 * T + s * 128
                for c in range(NCHUNK):
                    nc.tensor.matmul(
                        ps_v[:, g * H : (g + 1) * H],
                        lhsT=r(xt[:, c, t0 : t0 + 128]),
                        rhs=r(wv_sb[:, c, :]),
                        start=(c == 0),
                        stop=(c == NCHUNK - 1),
                    )
            v_sb = vaug[pr % NSLOT]
            nc.vector.tensor_copy(
                v_sb[:, :, 0:H],
                ps_v[:, 0:256].rearrange("p (g h) -> p g h", g=4),
            )

            ps_o = pso_pool.tile([128, 512], F32, tag="pso")
            for b2 in range(2):
                toff = b2 * T
                # wei blocks [s, t]: A = (s0, t0:256) via one N=256 matmul,
                # C = (s1, t1) via one N=128 matmul. (t<128 never sees s1.)
                ps_c = psc_pool.tile([128, 512], F32, tag="psc")
                nc.tensor.matmul(
                    ps_c[:, 0:256],
                    lhsT=r(k_sb[:, toff : toff + 128]),
                    rhs=r(q_sb[:, toff : toff + T]),
                    start=True,
                    stop=True,
                )
                nc.tensor.matmul(
                    ps_c[:, 256:384],
                    lhsT=r(k_sb[:, toff + 128 : toff + 256]),
                    rhs=r(q_sb[:, toff + 128 : toff + 256]),
                    start=True,
                    stop=True,
                )
                # P = exp(wei * scale), then causal mask [triu|ones|triu]
                p_sb = p_pool.tile([128, 384], F32R, tag="p")
                nc.scalar.activation(
                    p_sb[:], ps_c[:, 0:384],
                    mybir.ActivationFunctionType.Exp, scale=SCALE,
                )
                # causal mask on the two diagonal blocks (A at 0:128, C at
                # 256:384) in one strided DVE op
                pv = p_sb[:].rearrange("p (b x) -> p b x", x=128)[:, 0:3:2, :]
                nc.vector.tensor_mul(
                    pv, pv, mask_sb[:].rearrange("p (b x) -> p b x", x=128)
                )

                # out[t, 0:64] + denom[t] (col 64) via P-stationary matmuls
                g0 = 2 * b2
                nc.tensor.matmul(
                    ps_o[:, g0 * GW : g0 * GW + GW],
                    lhsT=r(p_sb[:, 0:128]),
                    rhs=r(v_sb[:, g0, :]),
                    start=True,
                    stop=True,
                )
                nc.tensor.matmul(
                    ps_o[:, (g0 + 1) * GW : (g0 + 2) * GW],
                    lhsT=r(p_sb[:, 128:256]),
                    rhs=r(v_sb[:, g0, :]),
                    start=True,
                    stop=False,
                )
                nc.tensor.matmul(
                    ps_o[:, (g0 + 1) * GW : (g0 + 2) * GW],
                    lhsT=r(p_sb[:, 256:384]),
                    rhs=r(v_sb[:, g0 + 1, :]),
                    start=False,
                    stop=True,
                )

            # normalize: out[t, h] / denom[t] for all 4 groups at once
            og = ps_o[:, 0 : 4 * GW].rearrange("p (g c) -> p g c", g=4)
            rs = r_pool.tile([128, 4], F32, tag="rs")
            rsv = rs[:].rearrange("p (g c) -> p g c", c=1)
            nc.vector.reciprocal(rsv, og[:, :, H : H + 1])
            out_sb = o_pool.tile([128, 4, H], F32, tag="out")
            in0, in1 = broadcast_tensor_aps(og[:, :, 0:H], rsv)
            nc.vector.tensor_mul(out_sb[:], in0, in1)
            nc.gpsimd.dma_start(outH[pr], out_sb[:])

    nc.finalize()
    return nc


def _host_inputs(x, Wq, Wk, Wv):
    B_, T_, C_ = x.shape
    assert (B_, T_, C_) == (B, T, C), (B_, T_, C_)
    # xh[pr, p, c, 256*b2 + t] = x[2*pr + b2, t, 128*c + p]
    xh = np.ascontiguousarray(
        x.reshape(B // 2, 2, T, NCHUNK, 128).transpose(0, 4, 3, 1, 2)
        .reshape(B // 2, 128, NCHUNK, 2 * T),
        dtype=np.float32,
    )
    wqk_h = np.ascontiguousarray(
        np.concatenate([Wq, Wk], axis=1).reshape(NCHUNK, 128, 128).transpose(1, 0, 2),
        dtype=np.float32,
    )
    wv_h = np.ascontiguousarray(
        Wv.reshape(NCHUNK, 128, H).transpose(1, 0, 2), dtype=np.float32
    )
    triu = np.triu(np.ones((128, 128), dtype=np.float32))
    mask_h = np.ascontiguousarray(np.concatenate([triu, triu], axis=1))
    return xh, wqk_h, wv_h, mask_h


def _make_in_maps(xh, wqk_h, wv_h, mask_h):
    return [
        {
            "xT": xh[i * NPAIR : (i + 1) * NPAIR],
            "wqk": wqk_h,
            "wv": wv_h,
            "mask": mask_h,
        }
        for i in range(N_CORES)
    ]


def _assemble(results):
    # outH per core: [NPAIR, 128, 256]; groups g = 2*b2 + thalf
    outH = np.concatenate([results[i]["outH"] for i in range(N_CORES)], axis=0)
    out = (
        outH.reshape(B // 2, 128, 2, 2, H)
        .transpose(0, 2, 3, 1, 4)
        .reshape(B, T, H)
    )
    return np.ascontiguousarray(out)


def kernel(x, Wq, Wk, Wv):
    x = np.asarray(x, dtype=np.float32)
    Wq = np.asarray(Wq, dtype=np.float32)
    Wk = np.asarray(Wk, dtype=np.float32)
    Wv = np.asarray(Wv, dtype=np.float32)

    xh, wqk_h, wv_h, mask_h = _host_inputs(x, Wq, Wk, Wv)
    nc = build_nc(NPAIR)
    in_maps = _make_in_maps(xh, wqk_h, wv_h, mask_h)
    res = run_bass_kernel_spmd(nc, in_maps, list(range(N_CORES)))
    return _assemble(res.results)


# revision 7
# speedup vs baseline: 3.0972x; 2.0078x over previous
"""Single-head causal self-attention on 8 Trainium2 NeuronCores (Bass/Tile).

Problem: x [1024, 256, 384], Wq/Wk/Wv [384, 64] ->
  q,k,v = x@W;  wei = softmax(mask(q k^T / sqrt(384)));  out = wei @ v
Output: [1024, 256, 64] fp32.

Strategy (data-parallel over batch, 128 batches = 64 pairs per core):
  - Layout-by-stationarity: every tensor is produced in exactly the layout
    its consumer needs, so there are NO transposes anywhere.
      q^T,k^T [h, t]  = W-stationary matmul   (lhsT = [Wq|Wk] chunk)
      v       [s, h]  = x-stationary matmul   (lhsT = x^T chunk)
      P       [s, t]  = k-stationary matmul   (lhsT = k^T s-half) + exp
      out     [t, h]  = P-stationary matmul   (lhsT = P block, rhs = [v|1])
  - The ones column appended to v makes the out-MM emit the softmax
    denominator per OUTPUT PARTITION (= token), so normalization is one
    cheap reciprocal [128, 4, 1] + one broadcast tensor_mul per pair.
  - Causal structure at 128-block granularity: tokens t<128 never touch
    s-half1, so wei needs only 3 of 4 blocks (A=s0xt0 diag, B=s0xt1 full,
    C=s1xt1 diag); exp covers [128, 384]; mask [triu|triu] on A,C only.
  - Pair batching: 2 batches per iteration; qk projection streams N=512,
    elementwise ops fused across the pair (recip FD=4, one norm op).
  - bf16 operand pipeline (fp32 PSUM accumulation): full-rate 1 elem/cycle
    PE streaming (fp32r is ~2 cyc/elem), FWL fast weight loads, half the
    HBM traffic and 2x DVE modes. Output stays fp32.
  - PSUM tiles padded to full banks (4 pools x 2 bufs = 8 banks exactly).
"""

import os
from contextlib import ExitStack

import ml_dtypes
import numpy as np

import concourse.bass as bass
import concourse.bacc as bacc
import concourse.tile as tile
from concourse import mybir
from concourse.bass import broadcast_tensor_aps
from concourse.bass_utils import run_bass_kernel_spmd

N_CORES = 8
B = 1024
T = 256
C = 384
H = 64
BPC = B // N_CORES  # 128 batches per core
NPAIR = BPC // 2  # 64 pairs per core
NCHUNK = C // 128  # 3
SCALE = float(C) ** -0.5
GW = H + 2  # [v | 1 | pad] group width (even free dim, 8B-aligned groups)

F32 = mybir.dt.float32
BF16 = mybir.dt.bfloat16
BF = ml_dtypes.bfloat16


def build_nc(npair: int = NPAIR):
    nc = bacc.Bacc(
        "TRN2", target_bir_lowering=False, debug=False, num_devices=N_CORES
    )

    # xT[pr, p, c, 256*b2 + t] = x[2*pr + b2, t, 128*c + p]
    xT = nc.dram_tensor("xT", [npair, 128, NCHUNK, 2 * T], BF16, kind="ExternalInput").ap()
    wqk = nc.dram_tensor("wqk", [128, NCHUNK, 128], BF16, kind="ExternalInput").ap()
    wv = nc.dram_tensor("wv", [128, NCHUNK, H], BF16, kind="ExternalInput").ap()
    # [triu | triu] for the A and C (diagonal) block columns of P
    mask = nc.dram_tensor("mask", [128, 256], BF16, kind="ExternalInput").ap()
    # outH[pr, p, 64*g + h] = out[2*pr + g//2, 128*(g%2) + p, h]
    outH = nc.dram_tensor("outH", [npair, 128, 4 * H], F32, kind="ExternalOutput").ap()

    with ExitStack() as ctx:
        tc = ctx.enter_context(tile.TileContext(nc))

        const = ctx.enter_context(tc.tile_pool(name="const", bufs=1))
        wqk_sb = const.tile([128, NCHUNK, 128], BF16, tag="wqk")
        nc.sync.dma_start(wqk_sb[:], wqk)
        wv_sb = const.tile([128, NCHUNK, H], BF16, tag="wv")
        nc.sync.dma_start(wv_sb[:], wv)
        mask_sb = const.tile([128, 256], BF16, tag="mask")
        nc.sync.dma_start(mask_sb[:], mask)

        # Persistent [v | 1 | pad] rhs tiles: 4 groups (b0s0, b0s1, b1s0,
        # b1s1), each [128, 66] with ones at cols 64:66.
        NSLOT = 3
        vaug = []
        for i in range(NSLOT):
            v_t = const.tile([128, 4, GW], BF16, tag=f"vaug{i}")
            nc.gpsimd.memset(v_t[:, :, H : H + 2], 1.0)
            vaug.append(v_t)

        xt_pool = ctx.enter_context(tc.tile_pool(name="xt", bufs=3))
        q_pool = ctx.enter_context(tc.tile_pool(name="qp", bufs=2))
        k_pool = ctx.enter_context(tc.tile_pool(name="kp", bufs=2))
        p_pool = ctx.enter_context(tc.tile_pool(name="pp", bufs=3))
        o_pool = ctx.enter_context(tc.tile_pool(name="op", bufs=2))
        r_pool = ctx.enter_context(tc.tile_pool(name="rp", bufs=2))

        psqk_pool = ctx.enter_context(tc.tile_pool(name="psqk", bufs=2, space="PSUM"))
        psv_pool = ctx.enter_context(tc.tile_pool(name="psv", bufs=2, space="PSUM"))
        psc_pool = ctx.enter_context(tc.tile_pool(name="psc", bufs=2, space="PSUM"))
        pso_pool = ctx.enter_context(tc.tile_pool(name="pso", bufs=2, space="PSUM"))

        for pr in range(npair):
            xt = xt_pool.tile([128, NCHUNK, 2 * T], BF16, tag="xt")
            nc.sync.dma_start(xt[:], xT[pr])

            # q^T|k^T for the pair: [128 qk-dims, 512 tokens]
            ps_qk = psqk_pool.tile([128, 512], F32, tag="psqk")
            for c in range(NCHUNK):
                nc.tensor.matmul(
                    ps_qk[:],
                    lhsT=wqk_sb[:, c, :],
                    rhs=xt[:, c, :],
                    start=(c == 0),
                    stop=(c == NCHUNK - 1),
                )
            q_sb = q_pool.tile([H, 512], BF16, tag="q")
            nc.scalar.copy(q_sb[:], ps_qk[0:H, :])
            k_sb = k_pool.tile([H, 512], BF16, tag="k")
            nc.scalar.copy(k_sb[:], ps_qk[H:128, :])

            # v [s, h] directly via x-stationary matmuls: group g = 2*b2 + s
            ps_v = psv_pool.tile([128, 512], F32, tag="psv")
            for g in range(4):
                b2, s = divmod(g, 2)
                t0 = b2 * T + s * 128
                for c in range(NCHUNK):
                    nc.tensor.matmul(
                        ps_v[:, g * H : (g + 1) * H],
                        lhsT=xt[:, c, t0 : t0 + 128],
                        rhs=wv_sb[:, c, :],
                        start=(c == 0),
                        stop=(c == NCHUNK - 1),
                    )
            v_sb = vaug[pr % NSLOT]
            nc.vector.tensor_copy(
                v_sb[:, :, 0:H],
                ps_v[:, 0:256].rearrange("p (g h) -> p g h", g=4),
            )

            ps_o = pso_pool.tile([128, 512], F32, tag="pso")
            for b2 in range(2):
                toff = b2 * T
                # wei blocks [s, t]: A|B = (s0, t0:256) via one N=256 matmul,
                # C = (s1, t1) via one N=128 matmul. (t<128 never sees s1.)
                ps_c = psc_pool.tile([128, 512], F32, tag="psc")
                nc.tensor.matmul(
                    ps_c[:, 0:256],
                    lhsT=k_sb[:, toff : toff + 128],
                    rhs=q_sb[:, toff : toff + T],
                    start=True,
                    stop=True,
                )
                nc.tensor.matmul(
                    ps_c[:, 256:384],
                    lhsT=k_sb[:, toff + 128 : toff + 256],
                    rhs=q_sb[:, toff + 128 : toff + 256],
                    start=True,
                    stop=True,
                )
                # P = exp(wei * scale), then causal mask on the two diagonal
                # blocks (A at 0:128, C at 256:384) in one strided DVE op
                p_sb = p_pool.tile([128, 384], BF16, tag="p")
                nc.scalar.activation(
                    p_sb[:], ps_c[:, 0:384],
                    mybir.ActivationFunctionType.Exp, scale=SCALE,
                )
                pv = p_sb[:].rearrange("p (b x) -> p b x", x=128)[:, 0:3:2, :]
                nc.vector.tensor_mul(
                    pv, pv, mask_sb[:].rearrange("p (b x) -> p b x", x=128)
                )

                # out[t, 0:64] + denom[t] (col 64) via P-stationary matmuls
                g0 = 2 * b2
                nc.tensor.matmul(
                    ps_o[:, g0 * GW : g0 * GW + GW],
                    lhsT=p_sb[:, 0:128],
                    rhs=v_sb[:, g0, :],
                    start=True,
                    stop=True,
                )
                nc.tensor.matmul(
                    ps_o[:, (g0 + 1) * GW : (g0 + 2) * GW],
                    lhsT=p_sb[:, 128:256],
                    rhs=v_sb[:, g0, :],
                    start=True,
                    stop=False,
                )
                nc.tensor.matmul(
                    ps_o[:, (g0 + 1) * GW : (g0 + 2) * GW],
                    lhsT=p_sb[:, 256:384],
                    rhs=v_sb[:, g0 + 1, :],
                    start=False,
                    stop=True,
                )

            # normalize: out[t, h] / denom[t] for all 4 groups at once
            og = ps_o[:, 0 : 4 * GW].rearrange("p (g c) -> p g c", g=4)
            rs = r_pool.tile([128, 4], F32, tag="rs")
            rsv = rs[:].rearrange("p (g c) -> p g c", c=1)
            nc.vector.reciprocal(rsv, og[:, :, H : H + 1])
            out_sb = o_pool.tile([128, 4, H], F32, tag="out")
            in0, in1 = broadcast_tensor_aps(og[:, :, 0:H], rsv)
            nc.vector.tensor_mul(out_sb[:], in0, in1)
            nc.gpsimd.dma_start(outH[pr], out_sb[:])

    nc.finalize()
    return nc


def _host_inputs(x, Wq, Wk, Wv):
    B_, T_, C_ = x.shape
    assert (B_, T_, C_) == (B, T, C), (B_, T_, C_)
    # xh[pr, p, c, 256*b2 + t] = x[2*pr + b2, t, 128*c + p]
    xh = np.ascontiguousarray(
        x.reshape(B // 2, 2, T, NCHUNK, 128).transpose(0, 4, 3, 1, 2)
        .reshape(B // 2, 128, NCHUNK, 2 * T)
        .astype(BF)
    )
    wqk_h = np.ascontiguousarray(
        np.concatenate([Wq, Wk], axis=1).reshape(NCHUNK, 128, 128).transpose(1, 0, 2)
        .astype(BF)
    )
    wv_h = np.ascontiguousarray(
        Wv.reshape(NCHUNK, 128, H).transpose(1, 0, 2).astype(BF)
    )
    triu = np.triu(np.ones((128, 128), dtype=BF))
    mask_h = np.ascontiguousarray(np.concatenate([triu, triu], axis=1))
    return xh, wqk_h, wv_h, mask_h


def _make_in_maps(xh, wqk_h, wv_h, mask_h):
    return [
        {
            "xT": xh[i * NPAIR : (i + 1) * NPAIR],
            "wqk": wqk_h,
            "wv": wv_h,
            "mask": mask_h,
        }
        for i in range(N_CORES)
    ]


def _assemble(results):
    # outH per core: [NPAIR, 128, 256]; groups g = 2*b2 + thalf
    outH = np.concatenate([results[i]["outH"] for i in range(N_CORES)], axis=0)
    out = (
        outH.reshape(B // 2, 128, 2, 2, H)
        .transpose(0, 2, 3, 1, 4)
        .reshape(B, T, H)
    )
    return np.ascontiguousarray(out)


def kernel(x, Wq, Wk, Wv):
    x = np.asarray(x, dtype=np.float32)
    Wq = np.asarray(Wq, dtype=np.float32)
    Wk = np.asarray(Wk, dtype=np.float32)
    Wv = np.asarray(Wv, dtype=np.float32)

    xh, wqk_h, wv_h, mask_h = _host_inputs(x, Wq, Wk, Wv)
    nc = build_nc(NPAIR)
    in_maps = _make_in_maps(xh, wqk_h, wv_h, mask_h)
    res = run_bass_kernel_spmd(nc, in_maps, list(range(N_CORES)))
    return _assemble(res.results)
